# revision 18
# baseline (speedup 1.0000x reference)
"""Multi-head attention (B=8, N=1024, C=768, H=12) on 8 TRN2 NeuronCores.

Sharding: pure data parallel — batch element b runs on core b. No collectives.

v2 design. The attention window (96 stages = pair x qh x ktile) is bounded
below by the ScalarE exp stream (~1us per [128,1024] stage) and the PE's
in-window work. Everything else is molded around keeping the PE saturated
slightly ABOVE the exp floor so the 2.4GHz p-state never drops (a PE idle
gap resets the clock ramp to 1.2GHz for 3us):

  1. x [1024,768] f32 -> PE-transpose (fp32) -> xT bf16 [768,1024].
  2. q,k projections TRANSPOSED (qkT[f,s] via lhsT=Wqk, rhs=xT). Only the
     nh=0 halves of pair-0's two tiles run up front; everything else
     (10.5 tiles) drains from a deadline-ordered filler deque inside the
     attention window.
  3. v computed HEAD-PAIR-major as filler: pair t+1's v is produced during
     pair t, so there is no v prefix. v is stored fp8(e4m3) as
     vsb8[ktile-pair] = [128, 2, H, 80] = [v | 1 | 0-pad]: the ones column
     folds the softmax denominator into the PV matmul (PSUM row 64); the
     pad reaches M=80 (DoubleRow needs M % 16 == 0, M=65 is rejected).
  4. scores^T [k,q] = kT.T @ qT, bf16, K=64 row-paired across the two heads
     (tile_position (0,0)/(64,0), ~1.6x concurrency on silicon).
  5. exp on ScalarE writes e4m3 DIRECTLY into 2-stage units
     pt8 = [128, 2, 1024]; PV runs every second stage as ONE fp8 DoubleRow
     matmul per head (K = 2x128 ktiles at 2x the bf16 ingest rate),
     accumulating outT[80, 512] over 4 steps (rows 0:64 = out, 64 = denom).
  6. normalize (deferred off critical path): PSUM->SBUF copy, reciprocal of
     row 64, partition broadcast via DRAM-bounce DMA (last pair: K=1
     ones-matmul on the PE instead), muls emitted two blocks later.
  7. proj: seq tiles 0..3 run as filler during pair-5 qh=1 (their aoT
     columns are final after the (5,0) normalize, whose deferred muls are
     flushed first); st 4..7 are the tail, emitted AFTER the next rep's
     prefix so the next window's exp stream starts during the tail.
  8. reps>1: next rep's x DMA is issued mid-body; its transposes execute
     during pair-5 qh=0 (which has no other filler supply left), keeping
     the PE hot there; Tile's WAR tracking on xT makes this safe.

PSUM (8 banks): 2 shared ring (qk/v/proj fills, transposes, broadcasts)
+ 4 scores (double-buffered [128,1024]) + 2 PV accumulators [80,512].
"""
import os
import sys

if "/opt/trn_rl_repo" not in sys.path:
    sys.path.insert(0, "/opt/trn_rl_repo")

PV_DR = os.environ.get("PV_DR", "1") == "1"

from contextlib import ExitStack

import numpy as np

import concourse.bass as bass
import concourse.tile as tile
from concourse import mybir
from concourse.bass_utils import run_bass_kernel_spmd
from concourse.masks import make_identity

FP32 = mybir.dt.float32
BF16 = mybir.dt.bfloat16
F8 = mybir.dt.float8e4
Exp = mybir.ActivationFunctionType.Exp
DR = mybir.MatmulPerfMode.DoubleRow

S = 1024          # sequence length (per core batch element)
C = 768           # model dim
H = 12            # heads
HD = 64           # head dim
C3 = 3 * C
P = 128
ST = S // P       # 8 seq tiles
CT = C // P       # 6 feature tiles
MT = 12           # q+k feature tiles of qkT
PAIRS = H // 2    # 6 head pairs
VM = 80           # DoubleRow lhsT M: 64 v dims + ones + 15 pad
SCALE = HD ** -0.5
N_CORES = 8

# Filler pacing: target emitted PE cycles per stage (even/odd = no-PV/PV).
QUOTA_EVEN = 1500
QUOTA_ODD = 1100


def split_multiwait(nc, max_waits=1):
    """This walrus build rejects instructions with >1 semaphore waits (the
    Tile kernel-tail Drain accumulates one per live proc). Split extras into
    chained Drains on the same engine immediately before."""
    for func in nc.m.functions:
        for block in func.blocks:
            newlist = []
            for ins in block.instructions:
                si = ins.sync_info
                if si is not None and si.on_wait is not None and len(si.on_wait) > max_waits:
                    waits = list(si.on_wait)
                    extra, keep = waits[:-max_waits], waits[-max_waits:]
                    for j, w in enumerate(extra):
                        nd = mybir.InstDrain(
                            name=f"{ins.name}-wsplit{j}",
                            engine=ins.engine,
                            ins=[], outs=[],
                            sync_info=mybir.SyncInfo(on_wait=[w], on_update=[]),
                        )
                        newlist.append(nd)
                        nc.inst_map[nd.name] = nd
                    ins.sync_info = mybir.SyncInfo(
                        on_wait=keep, on_update=list(si.on_update or [])
                    )
                newlist.append(ins)
            block.instructions = newlist


def build_nc(reps=1):
    nc = bass.Bass()
    x_ext = nc.declare_dram_parameter("x", [S, C], FP32, isOutput=False)
    qkvw_ext = nc.declare_dram_parameter("qkv_w", [C, C3], FP32, isOutput=False)
    qkvb_ext = nc.declare_dram_parameter("qkv_b", [C3], FP32, isOutput=False)
    projw_ext = nc.declare_dram_parameter("proj_w", [C, C], FP32, isOutput=False)
    projb_ext = nc.declare_dram_parameter("proj_b", [C], FP32, isOutput=False)
    out_ext = nc.declare_dram_parameter("out", [S, C], FP32, isOutput=True)

    with tile.TileContext(nc) as tc, ExitStack() as ctx:
        consts = ctx.enter_context(tc.tile_pool(name="consts", bufs=1))
        wpool = ctx.enter_context(tc.tile_pool(name="weights", bufs=1))
        xpool = ctx.enter_context(tc.tile_pool(name="xpool", bufs=1))
        actpool = ctx.enter_context(tc.tile_pool(name="actpool", bufs=1))
        ptpool = ctx.enter_context(tc.tile_pool(name="ptpool", bufs=2))
        rpool = ctx.enter_context(tc.tile_pool(name="rpool", bufs=1))
        opool = ctx.enter_context(tc.tile_pool(name="opool", bufs=2))
        stage = ctx.enter_context(tc.tile_pool(name="stage", bufs=2))
        dscr = ctx.enter_context(tc.tile_pool(name="dscr", bufs=2, space="DRAM"))
        ps_fill = ctx.enter_context(tc.tile_pool(name="ps_fill", bufs=2, space="PSUM"))
        ps_attn = ctx.enter_context(tc.tile_pool(name="ps_attn", bufs=1, space="PSUM"))

        def load_x(who=""):
            # 8 fine chunks: transposes can start after the first ~400KB.
            # bufs=1: rep r+1's DMA waits only rep r's prefix transposes.
            xf = stage.tile([P, ST, C], FP32, tag="xfall", bufs=1, name=f"xfall{who}")
            for st in range(ST):
                nc.gpsimd.dma_start(
                    out=xf[:, st, :],
                    in_=bass.AP(tensor=x_ext, offset=st * P * C,
                                ap=[[C, P], [1, C]]))
            return xf

        xf0 = load_x("0")

        # ---- constants / biases ----
        ident = consts.tile([P, P], FP32, tag="ident")
        make_identity(nc, ident)
        ones_r = consts.tile([P, HD], BF16, tag="ones_r")
        nc.vector.memset(ones_r[HD:HD + 1, :], 1.0)

        qkb = consts.tile([P, MT], FP32, tag="qkb")
        qkb_src = bass.AP(tensor=qkvb_ext, offset=0, ap=[[1, P], [P, MT]])
        nc.scalar.dma_start(out=qkb, in_=qkb_src)
        vb = consts.tile([P, C], FP32, tag="vb")
        vb_src = bass.AP(tensor=qkvb_ext, offset=2 * C, ap=[[0, P], [1, C]])
        nc.scalar.dma_start(out=vb, in_=vb_src)
        pb = consts.tile([P, C], FP32, tag="pb")
        pb_src = bass.AP(tensor=projb_ext, offset=0, ap=[[0, P], [1, C]])
        nc.scalar.dma_start(out=pb, in_=pb_src)

        # ---- load + cast weights (once) ----
        # qk columns: sync queue + ACT copies (both idle in the prefix);
        # v columns: scalar queue + Pool casts; proj: gpsimd queue, casts
        # deferred into the attention window via the filler deque.
        xT = [xpool.tile([P, S], BF16, tag=f"xT{ct}", name=f"xT{ct}") for ct in range(CT)]
        wqkv = [wpool.tile([P, C3], BF16, tag=f"wqkv{ct}", name=f"wqkv{ct}")
                for ct in range(CT)]
        # All weight staging shares one [P, 2, 768] fp32 ring (2 bufs).
        for ct in range(CT):
            stg = stage.tile([P, 2, C], FP32, tag="wstg", name=f"wstgq{ct}")
            nc.sync.dma_start(out=stg, in_=bass.AP(
                tensor=qkvw_ext, offset=ct * P * C3,
                ap=[[C3, P], [C, 2], [1, C]]))
            nc.scalar.copy(
                out=wqkv[ct][:, 0:2 * C].rearrange("p (a b) -> p a b", b=C),
                in_=stg)
        for chunk in range(3):
            cts = range(chunk * 2, chunk * 2 + 2)
            stgv = stage.tile([P, 2, C], FP32, tag="wstg", name=f"wstgv{chunk}")
            nc.scalar.dma_start(out=stgv, in_=bass.AP(
                tensor=qkvw_ext, offset=chunk * 2 * P * C3 + 2 * C,
                ap=[[C3, P], [P * C3, 2], [1, C]]))
            for j, ct in enumerate(cts):
                nc.gpsimd.tensor_copy(out=wqkv[ct][:, 2 * C:], in_=stgv[:, j, :])
        # proj: DMA + cast deferred into the attention window (filler steps).
        # Staging tiles must stay live until the casts: dedicated ring slots.
        wproj = [wpool.tile([P, C], BF16, tag=f"wproj{ct}", name=f"wproj{ct}")
                 for ct in range(CT)]
        pstg = []
        for chunk in range(3):
            stgp = stage.tile([P, 2, C], FP32, tag=f"pstg{chunk}",
                              name=f"pstg{chunk}", bufs=1)
            nc.gpsimd.dma_start(out=stgp, in_=bass.AP(
                tensor=projw_ext, offset=chunk * 2 * P * C,
                ap=[[C, P], [P * C, 2], [1, C]]))
            pstg.append(stgp)

        # ---- persistent activation tiles ----
        qk = [actpool.tile([P, S], BF16, tag=f"qk{mt}", name=f"qk{mt}") for mt in range(MT)]
        # fp8 v: [kpos, ktile-half, head, v|1|pad]
        vsb8 = [actpool.tile([P, 2, H, VM], F8 if PV_DR else BF16,
                             tag=f"v8_{sp}", name=f"v8_{sp}")
                for sp in range(ST // 2)]
        for sp in range(ST // 2):
            nc.vector.memset(vsb8[sp][:, :, :, HD:HD + 1], 1.0)
            nc.vector.memset(vsb8[sp][:, :, :, HD + 1:VM], 0.0)
        aoT = [actpool.tile([P, S], BF16, tag=f"aoT{t}", name=f"aoT{t}") for t in range(PAIRS)]

        # =================== filler steps ===================
        # Each step: (est_cycles, fn). Correctness never depends on pacing —
        # Tile syncs all data deps; pacing only shapes the PE timeline.

        def transpose_steps(xf, use_act):
            steps = []
            for sg in range(2):
                for ct in range(CT):
                    def fn(sg=sg, ct=ct, xf=xf, use_act=use_act):
                        tps = ps_fill.tile([P, 512], FP32, tag="qkps", name=f"tps{sg}_{ct}")
                        for j in range(4):
                            st = sg * 4 + j
                            nc.tensor.transpose(
                                out=tps[:, j * P:(j + 1) * P],
                                in_=xf[:, st, ct * P:(ct + 1) * P],
                                identity=ident,
                            )
                        if use_act:
                            nc.scalar.copy(
                                out=xT[ct][:, sg * 512:(sg + 1) * 512], in_=tps)
                        else:
                            nc.vector.tensor_copy(
                                out=xT[ct][:, sg * 512:(sg + 1) * 512], in_=tps)
                    steps.append((10 ** 6, 1100, fn))
            return steps

        def qk_half_steps(mt, nh, deadline=10 ** 6):
            """6 accumulation matmuls producing qk[mt] columns nh*512:+512.

            Tile links reads only to PRIOR writes, so every filler carries a
            hard deadline: the stage index before whose emission it MUST have
            been emitted (consumers emitted later then get real RAW edges).
            """
            steps = []
            box = {}
            for ct in range(CT):
                def fn(mt=mt, nh=nh, ct=ct, box=box):
                    if ct == 0:
                        box["ps"] = ps_fill.tile([P, 512], FP32, tag="qkps",
                                                 name=f"qkps{mt}_{nh}")
                    ps = box["ps"]
                    nc.tensor.matmul(
                        ps,
                        lhsT=wqkv[ct][:, mt * P:(mt + 1) * P],
                        rhs=xT[ct][:, nh * 512:(nh + 1) * 512],
                        start=(ct == 0), stop=(ct == CT - 1),
                    )
                    if ct == CT - 1:
                        nc.vector.tensor_scalar_add(
                            out=qk[mt][:, nh * 512:(nh + 1) * 512],
                            in0=ps, scalar1=qkb[:, mt:mt + 1],
                        )
                steps.append((deadline, 512, fn))
            return steps

        def v_steps(t, base=0):
            """v for head pair t (dims t*128..+128), all 8 seq tiles.
            4 seq tiles share one [P,512] PSUM tile (one quarter each).
            Deadline: PV(t, 0, kk) consumes vsb8[kk//2] head slices at stage
            base + kk."""
            steps = []
            box = {}
            for st in range(ST):
                def fn(t=t, st=st, box=box):
                    q = st % 4
                    if q == 0:
                        box["ps"] = ps_fill.tile([P, 512], FP32, tag="qkps",
                                                 name=f"vps{t}_{st}")
                    ps = box["ps"]
                    sl = slice(q * P, (q + 1) * P)
                    for ct in range(CT):
                        nc.tensor.matmul(
                            ps[:, sl],
                            lhsT=xT[ct][:, st * P:(st + 1) * P],
                            rhs=wqkv[ct][:, 2 * C + t * P: 2 * C + (t + 1) * P],
                            start=(ct == 0), stop=(ct == CT - 1),
                        )
                    nc.vector.tensor_add(
                        out=vsb8[st // 2][:, st % 2, 2 * t:2 * t + 2, 0:HD],
                        in0=ps[:, sl].rearrange("p (h d) -> p h d", d=HD),
                        in1=vb[:, t * P:(t + 1) * P].rearrange("p (h d) -> p h d", d=HD),
                    )
                steps.append((base + ((st | 1) if PV_DR else st), 800, fn))
            return steps

        def v_wide_steps(u, base):
            """v for pairs 2u,2u+1 together: 256-col streams halve the
            per-128-col Ldweights overhead; only for pairs 2..5 whose
            deadlines are far enough to absorb the coarser steps."""
            steps = []
            box = {}
            for st in range(ST):
                def fn(u=u, st=st, box=box):
                    if st % 2 == 0:
                        box["ps"] = ps_fill.tile([P, 512], FP32, tag="qkps",
                                                 name=f"vwps{u}_{st}")
                    ps = box["ps"]
                    sl = slice((st % 2) * 256, (st % 2) * 256 + 256)
                    for ct in range(CT):
                        nc.tensor.matmul(
                            ps[:, sl],
                            lhsT=xT[ct][:, st * P:(st + 1) * P],
                            rhs=wqkv[ct][:, 2 * C + u * 256: 2 * C + (u + 1) * 256],
                            start=(ct == 0), stop=(ct == CT - 1),
                        )
                    nc.vector.tensor_add(
                        out=vsb8[st // 2][:, st % 2, 4 * u:4 * u + 4, 0:HD],
                        in0=ps[:, sl].rearrange("p (h d) -> p h d", d=HD),
                        in1=vb[:, u * 256:(u + 1) * 256].rearrange("p (h d) -> p h d", d=HD),
                    )
                steps.append((base + ((st | 1) if PV_DR else st), 1600, fn))
            return steps

        def pcast_steps(deadline):
            steps = []
            for ct in range(CT):
                def fn(ct=ct):
                    nc.vector.tensor_copy(out=wproj[ct], in_=pstg[ct // 2][:, ct % 2, :])
                steps.append((deadline, 50, fn))
            return steps

        def proj_st_steps(st):
            """proj for seq tile st in column halves of 512 and 256."""
            steps = []
            box = {}
            for half, width in ((0, 512), (1, 256)):
                for ct in range(CT):
                    def fn(st=st, half=half, width=width, ct=ct, box=box):
                        off = half * 512
                        if ct == 0:
                            box[half] = ps_fill.tile([P, 512], FP32, tag="qkps",
                                                     name=f"prps{st}_{half}")
                        ps = box[half]
                        nc.tensor.matmul(
                            ps[:, 0:width],
                            lhsT=aoT[ct][:, st * P:(st + 1) * P],
                            rhs=wproj[ct][:, off:off + width],
                            start=(ct == 0), stop=(ct == CT - 1),
                        )
                        if ct == CT - 1:
                            ost = opool.tile([P, 512], FP32, tag="ostg", bufs=3,
                                             name=f"ostg{st}_{half}")
                            nc.vector.tensor_add(
                                out=ost[:, 0:width], in0=ps[:, 0:width],
                                in1=pb[:, off:off + width])
                            eng = (nc.scalar, nc.sync, nc.gpsimd)[(2 * st + half) % 3]
                            eng.dma_start(
                                out=bass.AP(tensor=out_ext,
                                            offset=st * P * C + off,
                                            ap=[[C, P], [1, width]]),
                                in_=ost[:, 0:width])
                    steps.append((10 ** 6, width, fn))
            return steps

        # =================== per-rep body ===================
        prev_tail = None
        xf_next = None
        for _rep in range(reps):
            first = _rep == 0
            last = _rep == reps - 1
            xf = xf0 if first else xf_next

            if first:
                for _, _, fn in transpose_steps(xf, use_act=True):
                    fn()
            if not last:
                xf_next = load_x(f"{_rep + 1}")

            # prefix: ONLY what gates scores(0)/exp(0) — pair-0's nh0 qk
            # fills. Everything else rides the deadline-sorted deque.
            vs0 = v_steps(0, base=0)
            for _, _, fn in qk_half_steps(0, 0) + qk_half_steps(PAIRS, 0):
                fn()

            # filler deque: (deadline_stage, cost, fn), consumed via pop()
            fillers = []
            fillers += vs0
            if prev_tail is not None:
                # previous rep's proj st4..7: early filler so the new exp
                # stream overlaps the old tail. Deadline 20 keeps it from
                # stalling this rep's deferred aoT writes (WAR edges).
                fillers += [(20, c, f) for _, c, f in prev_tail]
                prev_tail = None
            fillers += qk_half_steps(PAIRS, 1, deadline=3)
            fillers += qk_half_steps(0, 1, deadline=7)
            if first:
                fillers += pcast_steps(deadline=87)
            for t2 in range(1, PAIRS):
                base = 16 * t2
                a = (qk_half_steps(t2, 0, deadline=base - 1)
                     + qk_half_steps(t2, 1, deadline=base + 7))
                b = (qk_half_steps(PAIRS + t2, 0, deadline=base - 1)
                     + qk_half_steps(PAIRS + t2, 1, deadline=base + 3))
                if t2 == 1:
                    v = v_steps(t2, base=base)
                elif t2 in (2, 4):
                    v = v_wide_steps(t2 // 2, base=base)
                else:
                    v = []
                merged = []
                for i in range(12):
                    if i < len(v):
                        merged.append(v[i])
                    merged.append(a[i])
                    merged.append(b[i])
                fillers += merged
            fillers.sort(key=lambda s: s[0])
            fillers.reverse()

            # pair-5 qh=0 PE feed: the next rep's transposes
            tp_next = transpose_steps(xf_next, use_act=False) if not last else []
            tp_next.reverse()

            # proj fillers: only during pair-5 qh=1 stages
            proj_window = []
            for st in range(4):
                proj_window += proj_st_steps(st)
            proj_window.reverse()

            pending_muls = []
            if True:
                stages = [(t, qh, kk)
                          for t in range(PAIRS) for qh in range(2) for kk in range(ST)]
                sab_tiles = {}
                pt8_tiles = {}

                def emit_scores(s):
                    t, qh, kk = s
                    qsl = slice(qh * 512, (qh + 1) * 512)
                    ksl = slice(kk * P, (kk + 1) * P)
                    sAB = ps_attn.tile([P, S], FP32, tag="sAB", bufs=2,
                                       name=f"sAB{t}_{qh}_{kk}")
                    nc.tensor.matmul(
                        sAB[:, 0:512],
                        lhsT=qk[PAIRS + t][0:HD, ksl], rhs=qk[t][0:HD, qsl],
                        start=True, stop=True, tile_position=(0, 0),
                    )
                    nc.tensor.matmul(
                        sAB[:, 512:1024],
                        lhsT=qk[PAIRS + t][HD:P, ksl], rhs=qk[t][HD:P, qsl],
                        start=True, stop=True, tile_position=(HD, 0),
                    )
                    sab_tiles[s] = sAB

                emit_scores(stages[0])
                oA = oB = None
                for i, s in enumerate(stages):
                    t, qh, kk = s
                    last_pair = t == PAIRS - 1
                    # hard deadlines: anything this stage's emissions consume
                    # must already be emitted (Tile links reads only to prior
                    # writes — a late filler is a CORRECTNESS bug).
                    while fillers and fillers[-1][0] <= i:
                        fillers.pop()[2]()
                    sAB = sab_tiles.pop(s)
                    if kk % 2 == 0:
                        pt8 = ptpool.tile([P, 2, S], F8 if PV_DR else BF16,
                                          tag="pt8", bufs=3 if PV_DR else 2,
                                          name=f"pt8_{t}_{qh}_{kk}")
                        pt8_tiles[(t, qh, kk // 2)] = pt8
                    else:
                        pt8 = pt8_tiles.pop((t, qh, kk // 2))
                    nc.scalar.activation(
                        out=pt8[:, kk % 2, :], in_=sAB, func=Exp, scale=SCALE)
                    if i + 1 < len(stages):
                        emit_scores(stages[i + 1])
                    if PV_DR and kk % 2 == 1:
                        if kk == 1:
                            oA = ps_attn.tile([VM, 512], FP32, tag="oA", name=f"oA{t}_{qh}")
                            oB = ps_attn.tile([VM, 512], FP32, tag="oB", name=f"oB{t}_{qh}")
                        sp = kk // 2
                        nc.tensor.matmul(
                            oA, lhsT=vsb8[sp][:, :, 2 * t, :], rhs=pt8[:, :, 0:512],
                            start=(kk == 1), stop=(kk == ST - 1), perf_mode=DR,
                        )
                        nc.tensor.matmul(
                            oB, lhsT=vsb8[sp][:, :, 2 * t + 1, :], rhs=pt8[:, :, 512:1024],
                            start=(kk == 1), stop=(kk == ST - 1), perf_mode=DR,
                        )
                    elif not PV_DR:
                        if kk == 0:
                            oA = ps_attn.tile([VM, 512], FP32, tag="oA", name=f"oA{t}_{qh}")
                            oB = ps_attn.tile([VM, 512], FP32, tag="oB", name=f"oB{t}_{qh}")
                        sp = kk // 2
                        nc.tensor.matmul(
                            oA, lhsT=vsb8[sp][:, kk % 2, 2 * t, :], rhs=pt8[:, kk % 2, 0:512],
                            start=(kk == 0), stop=(kk == ST - 1),
                        )
                        nc.tensor.matmul(
                            oB, lhsT=vsb8[sp][:, kk % 2, 2 * t + 1, :], rhs=pt8[:, kk % 2, 512:1024],
                            start=(kk == 0), stop=(kk == ST - 1),
                        )
                    # drain fillers up to the stage quota; pair-5 has no
                    # later deadlines — drain its feeds (transposes/proj)
                    # aggressively so nothing spills past the window
                    if last_pair:
                        quota = 2000 if qh == 0 else 2600
                    else:
                        quota = QUOTA_ODD if kk % 2 else QUOTA_EVEN
                    acc = 0
                    while acc < quota:
                        if fillers:
                            src = fillers
                        elif last_pair and qh == 0 and tp_next:
                            src = tp_next
                        elif last_pair and qh == 1 and proj_window:
                            src = proj_window
                        else:
                            break
                        _, cost, fn = src.pop()
                        fn()
                        acc += cost

                    if kk != ST - 1:
                        continue
                    # (t, qh) slice complete: fast PSUM release + deferred norm
                    qsl = slice(qh * 512, (qh + 1) * 512)
                    uA = rpool.tile([P, 512], FP32, tag="uA", bufs=3)
                    uB = rpool.tile([P, 512], FP32, tag="uB", bufs=3)
                    nc.vector.tensor_copy(out=uA[0:HD + 1, :], in_=oA[0:HD + 1, :])
                    nc.vector.tensor_copy(out=uB[0:HD + 1, :], in_=oB[0:HD + 1, :])
                    uBs = rpool.tile([P, 512], FP32, tag="uBs", bufs=3)
                    if last_pair:
                        # fast-path: broadcast via K=1 PE matmul, not DMA
                        nc.vector.reciprocal(out=uA[HD:HD + 1, :], in_=uA[HD:HD + 1, :])
                        nc.vector.reciprocal(out=uB[HD:HD + 1, :], in_=uB[HD:HD + 1, :])
                        nc.sync.dma_start(out=uBs[HD:P, :], in_=uB[0:HD, :])
                        rbf = rpool.tile([P, 512], BF16, tag="rbf", bufs=1)
                        nc.vector.tensor_copy(out=rbf[HD:HD + 1, :], in_=uA[HD:HD + 1, :])
                        rbfB = rpool.tile([P, 512], BF16, tag="rbfB", bufs=1)
                        nc.vector.tensor_copy(out=rbfB[HD:HD + 1, :], in_=uB[HD:HD + 1, :])
                        rAp = ps_fill.tile([HD, 512], FP32, tag="qkps", name=f"rAp{qh}")
                        nc.tensor.matmul(
                            rAp, lhsT=ones_r[HD:HD + 1, 0:HD], rhs=rbf[HD:HD + 1, :],
                            start=True, stop=True, tile_position=(HD, 0),
                        )
                        rBp = ps_fill.tile([P, 512], FP32, tag="qkps", name=f"rBp{qh}")
                        nc.tensor.matmul(
                            rBp[HD:P, :], lhsT=ones_r[HD:HD + 1, 0:HD], rhs=rbfB[HD:HD + 1, :],
                            start=True, stop=True, tile_position=(HD, HD),
                        )
                        nc.vector.tensor_mul(
                            out=aoT[t][0:HD, qsl], in0=uA[0:HD, :], in1=rAp[0:HD, :]
                        )
                        nc.vector.tensor_mul(
                            out=aoT[t][HD:P, qsl], in0=uBs[HD:P, :], in1=rBp[HD:P, :]
                        )
                        if qh == 0:
                            # proj fillers read aoT[0..4] next: flush deferred muls
                            for m in pending_muls:
                                m()
                            pending_muls.clear()
                    else:
                        nc.vector.reciprocal(out=uA[HD:HD + 1, :], in_=uA[HD:HD + 1, :])
                        nc.vector.reciprocal(out=uB[HD:HD + 1, :], in_=uB[HD:HD + 1, :])
                        nc.gpsimd.dma_start(out=uBs[HD:P, :], in_=uB[0:HD, :])
                        rA = rpool.tile([P, 512], FP32, tag="rA", bufs=3)
                        rB = rpool.tile([P, 512], FP32, tag="rB", bufs=3)
                        dA = dscr.tile([512], FP32, tag="dA")
                        dB = dscr.tile([512], FP32, tag="dB")
                        nc.sync.dma_start(out=dA, in_=uA[HD:HD + 1, :])
                        nc.gpsimd.dma_start(out=dB, in_=uB[HD:HD + 1, :])
                        nc.sync.dma_start(
                            out=rA[0:HD, :],
                            in_=bass.AP(tensor=dA.tensor, offset=dA.offset, ap=[[0, HD], [1, 512]]),
                        )
                        nc.gpsimd.dma_start(
                            out=rB[HD:P, :],
                            in_=bass.AP(tensor=dB.tensor, offset=dB.offset, ap=[[0, HD], [1, 512]]),
                        )
                        def emit_muls(t=t, qsl=qsl, uA=uA, rA=rA, uBs=uBs, rB=rB):
                            nc.vector.tensor_mul(
                                out=aoT[t][0:HD, qsl], in0=uA[0:HD, :], in1=rA[0:HD, :]
                            )
                            nc.vector.tensor_mul(
                                out=aoT[t][HD:P, qsl], in0=uBs[HD:P, :], in1=rB[HD:P, :]
                            )
                        pending_muls.append(emit_muls)
                        if len(pending_muls) > 2:
                            pending_muls.pop(0)()

            for m in pending_muls:
                m()
            pending_muls.clear()
            # leftovers (if quotas underdrained — correctness, not perf)
            for lst in (fillers, tp_next, proj_window):
                while lst:
                    lst.pop()[2]()

            prev_tail = []
            for st in range(4, ST):
                prev_tail += proj_st_steps(st)

        for _, _, fn in prev_tail:
            fn()

    split_multiwait(nc)
    return nc


_NC_CACHE = None


def get_nc():
    global _NC_CACHE
    if _NC_CACHE is None:
        _NC_CACHE = build_nc()
    return _NC_CACHE


def kernel(x, qkv_w, qkv_b, proj_w, proj_b):
    x = np.ascontiguousarray(np.asarray(x, dtype=np.float32))
    in_common = {
        "qkv_w": np.ascontiguousarray(np.asarray(qkv_w, dtype=np.float32)),
        "qkv_b": np.ascontiguousarray(np.asarray(qkv_b, dtype=np.float32)),
        "proj_w": np.ascontiguousarray(np.asarray(proj_w, dtype=np.float32)),
        "proj_b": np.ascontiguousarray(np.asarray(proj_b, dtype=np.float32)),
    }
    in_maps = [{"x": x[b], **in_common} for b in range(N_CORES)]
    nc = get_nc()
    res = run_bass_kernel_spmd(nc, in_maps, core_ids=list(range(N_CORES)))
    return np.stack([res.results[b]["out"] for b in range(N_CORES)], axis=0)


# revision 19
# speedup vs baseline: 1.3776x; 1.3776x over previous
"""Multi-head attention (B=8, N=1024, C=768, H=12) on 8 TRN2 NeuronCores.

Sharding: pure data parallel — batch element b runs on core b. No collectives.

v2 design. The attention window (96 stages = pair x qh x ktile) is bounded
below by the ScalarE exp stream (~1us per [128,1024] stage) and the PE's
in-window work. Everything else is molded around keeping the PE saturated
slightly ABOVE the exp floor so the 2.4GHz p-state never drops (a PE idle
gap resets the clock ramp to 1.2GHz for 3us):

  1. x [1024,768] f32 -> PE-transpose (fp32) -> xT bf16 [768,1024].
  2. q,k projections TRANSPOSED (qkT[f,s] via lhsT=Wqk, rhs=xT). Only the
     nh=0 halves of pair-0's two tiles run up front; everything else
     (10.5 tiles) drains from a deadline-ordered filler deque inside the
     attention window.
  3. v computed HEAD-PAIR-major as filler: pair t+1's v is produced during
     pair t, so there is no v prefix. v is stored fp8(e4m3) as
     vsb8[ktile-pair] = [128, 2, H, 80] = [v | 1 | 0-pad]: the ones column
     folds the softmax denominator into the PV matmul (PSUM row 64); the
     pad reaches M=80 (DoubleRow needs M % 16 == 0, M=65 is rejected).
  4. scores^T [k,q] = kT.T @ qT, bf16, K=64 row-paired across the two heads
     (tile_position (0,0)/(64,0), ~1.6x concurrency on silicon).
  5. exp on ScalarE writes e4m3 DIRECTLY into 2-stage units
     pt8 = [128, 2, 1024]; PV runs every second stage as ONE fp8 DoubleRow
     matmul per head (K = 2x128 ktiles at 2x the bf16 ingest rate),
     accumulating outT[80, 512] over 4 steps (rows 0:64 = out, 64 = denom).
  6. normalize (deferred off critical path): PSUM->SBUF copy, reciprocal of
     row 64, partition broadcast via DRAM-bounce DMA (last pair: K=1
     ones-matmul on the PE instead), muls emitted two blocks later.
  7. proj: seq tiles 0..3 run as filler during pair-5 qh=1 (their aoT
     columns are final after the (5,0) normalize, whose deferred muls are
     flushed first); st 4..7 are the tail, emitted AFTER the next rep's
     prefix so the next window's exp stream starts during the tail.
  8. reps>1: next rep's x DMA is issued mid-body; its transposes execute
     during pair-5 qh=0 (which has no other filler supply left), keeping
     the PE hot there; Tile's WAR tracking on xT makes this safe.

PSUM (8 banks): 2 shared ring (qk/v/proj fills, transposes, broadcasts)
+ 4 scores (double-buffered [128,1024]) + 2 PV accumulators [80,512].
"""
import os
import sys

if "/opt/trn_rl_repo" not in sys.path:
    sys.path.insert(0, "/opt/trn_rl_repo")

PV_DR = os.environ.get("PV_DR", "1") == "1"

from contextlib import ExitStack

import numpy as np

import concourse.bass as bass
import concourse.tile as tile
from concourse import mybir
from concourse.bass_utils import run_bass_kernel_spmd
from concourse.masks import make_identity

FP32 = mybir.dt.float32
BF16 = mybir.dt.bfloat16
F8 = mybir.dt.float8e4
Exp = mybir.ActivationFunctionType.Exp
DR = mybir.MatmulPerfMode.DoubleRow

S = 1024          # sequence length (per core batch element)
C = 768           # model dim
H = 12            # heads
HD = 64           # head dim
C3 = 3 * C
P = 128
ST = S // P       # 8 seq tiles
CT = C // P       # 6 feature tiles
MT = 12           # q+k feature tiles of qkT
PAIRS = H // 2    # 6 head pairs
VM = 80           # DoubleRow lhsT M: 64 v dims + ones + 15 pad
SCALE = HD ** -0.5
N_CORES = 8

# Filler pacing: target emitted PE cycles per stage (even/odd = no-PV/PV).
QUOTA_EVEN = 1500
QUOTA_ODD = 1100


def split_multiwait(nc, max_waits=1):
    """This walrus build rejects instructions with >1 semaphore waits (the
    Tile kernel-tail Drain accumulates one per live proc). Split extras into
    chained Drains on the same engine immediately before."""
    for func in nc.m.functions:
        for block in func.blocks:
            newlist = []
            for ins in block.instructions:
                si = ins.sync_info
                if si is not None and si.on_wait is not None and len(si.on_wait) > max_waits:
                    waits = list(si.on_wait)
                    extra, keep = waits[:-max_waits], waits[-max_waits:]
                    for j, w in enumerate(extra):
                        nd = mybir.InstDrain(
                            name=f"{ins.name}-wsplit{j}",
                            engine=ins.engine,
                            ins=[], outs=[],
                            sync_info=mybir.SyncInfo(on_wait=[w], on_update=[]),
                        )
                        newlist.append(nd)
                        nc.inst_map[nd.name] = nd
                    ins.sync_info = mybir.SyncInfo(
                        on_wait=keep, on_update=list(si.on_update or [])
                    )
                newlist.append(ins)
            block.instructions = newlist


def build_nc(reps=1):
    nc = bass.Bass()
    x_ext = nc.declare_dram_parameter("x", [S, C], FP32, isOutput=False)
    qkvw_ext = nc.declare_dram_parameter("qkv_w", [C, C3], FP32, isOutput=False)
    qkvb_ext = nc.declare_dram_parameter("qkv_b", [C3], FP32, isOutput=False)
    projw_ext = nc.declare_dram_parameter("proj_w", [C, C], FP32, isOutput=False)
    projb_ext = nc.declare_dram_parameter("proj_b", [C], FP32, isOutput=False)
    out_ext = nc.declare_dram_parameter("out", [S, C], FP32, isOutput=True)

    with tile.TileContext(nc) as tc, ExitStack() as ctx:
        consts = ctx.enter_context(tc.tile_pool(name="consts", bufs=1))
        wpool = ctx.enter_context(tc.tile_pool(name="weights", bufs=1))
        xpool = ctx.enter_context(tc.tile_pool(name="xpool", bufs=1))
        actpool = ctx.enter_context(tc.tile_pool(name="actpool", bufs=1))
        ptpool = ctx.enter_context(tc.tile_pool(name="ptpool", bufs=2))
        rpool = ctx.enter_context(tc.tile_pool(name="rpool", bufs=1))
        opool = ctx.enter_context(tc.tile_pool(name="opool", bufs=2))
        stage = ctx.enter_context(tc.tile_pool(name="stage", bufs=2))
        dscr = ctx.enter_context(tc.tile_pool(name="dscr", bufs=2, space="DRAM"))
        ps_fill = ctx.enter_context(tc.tile_pool(name="ps_fill", bufs=2, space="PSUM"))
        ps_attn = ctx.enter_context(tc.tile_pool(name="ps_attn", bufs=1, space="PSUM"))

        def load_x(who=""):
            # 8 fine chunks: transposes can start after the first ~400KB.
            # bufs=1: rep r+1's DMA waits only rep r's prefix transposes.
            xf = stage.tile([P, ST, C], FP32, tag="xfall", bufs=1, name=f"xfall{who}")
            for st in range(ST):
                nc.gpsimd.dma_start(
                    out=xf[:, st, :],
                    in_=bass.AP(tensor=x_ext, offset=st * P * C,
                                ap=[[C, P], [1, C]]))
            return xf

        xf0 = load_x("0")

        # ---- constants / biases ----
        ident = consts.tile([P, P], FP32, tag="ident")
        make_identity(nc, ident)
        ones_r = consts.tile([P, HD], BF16, tag="ones_r")
        nc.vector.memset(ones_r[HD:HD + 1, :], 1.0)

        qkb = consts.tile([P, MT], FP32, tag="qkb")
        qkb_src = bass.AP(tensor=qkvb_ext, offset=0, ap=[[1, P], [P, MT]])
        nc.scalar.dma_start(out=qkb, in_=qkb_src)
        vb = consts.tile([P, C], FP32, tag="vb")
        vb_src = bass.AP(tensor=qkvb_ext, offset=2 * C, ap=[[0, P], [1, C]])
        nc.scalar.dma_start(out=vb, in_=vb_src)
        pb = consts.tile([P, C], FP32, tag="pb")
        pb_src = bass.AP(tensor=projb_ext, offset=0, ap=[[0, P], [1, C]])
        nc.scalar.dma_start(out=pb, in_=pb_src)

        # ---- load + cast weights (once) ----
        # qk columns: sync queue + ACT copies (both idle in the prefix);
        # v columns: scalar queue + Pool casts; proj: gpsimd queue, casts
        # deferred into the attention window via the filler deque.
        xT = [xpool.tile([P, S], BF16, tag=f"xT{ct}", name=f"xT{ct}") for ct in range(CT)]
        wqkv = [wpool.tile([P, C3], BF16, tag=f"wqkv{ct}", name=f"wqkv{ct}")
                for ct in range(CT)]
        # All weight staging shares one [P, 2, 768] fp32 ring (2 bufs).
        for ct in range(CT):
            stg = stage.tile([P, 2, C], FP32, tag="wstg", name=f"wstgq{ct}")
            nc.sync.dma_start(out=stg, in_=bass.AP(
                tensor=qkvw_ext, offset=ct * P * C3,
                ap=[[C3, P], [C, 2], [1, C]]))
            nc.scalar.copy(
                out=wqkv[ct][:, 0:2 * C].rearrange("p (a b) -> p a b", b=C),
                in_=stg)
        for chunk in range(3):
            cts = range(chunk * 2, chunk * 2 + 2)
            stgv = stage.tile([P, 2, C], FP32, tag="wstg", name=f"wstgv{chunk}")
            nc.scalar.dma_start(out=stgv, in_=bass.AP(
                tensor=qkvw_ext, offset=chunk * 2 * P * C3 + 2 * C,
                ap=[[C3, P], [P * C3, 2], [1, C]]))
            for j, ct in enumerate(cts):
                nc.gpsimd.tensor_copy(out=wqkv[ct][:, 2 * C:], in_=stgv[:, j, :])
        # proj: DMA + cast deferred into the attention window (filler steps).
        # Staging tiles must stay live until the casts: dedicated ring slots.
        wproj = [wpool.tile([P, C], BF16, tag=f"wproj{ct}", name=f"wproj{ct}")
                 for ct in range(CT)]
        pstg = []
        for chunk in range(3):
            stgp = stage.tile([P, 2, C], FP32, tag=f"pstg{chunk}",
                              name=f"pstg{chunk}", bufs=1)
            nc.gpsimd.dma_start(out=stgp, in_=bass.AP(
                tensor=projw_ext, offset=chunk * 2 * P * C,
                ap=[[C, P], [P * C, 2], [1, C]]))
            pstg.append(stgp)

        # ---- persistent activation tiles ----
        qk = [actpool.tile([P, S], BF16, tag=f"qk{mt}", name=f"qk{mt}") for mt in range(MT)]
        # fp8 v: [kpos, ktile-half, head, v|1|pad]
        vsb8 = [actpool.tile([P, 2, H, VM], F8 if PV_DR else BF16,
                             tag=f"v8_{sp}", name=f"v8_{sp}")
                for sp in range(ST // 2)]
        for sp in range(ST // 2):
            nc.vector.memset(vsb8[sp][:, :, :, HD:HD + 1], 1.0)
            nc.vector.memset(vsb8[sp][:, :, :, HD + 1:VM], 0.0)
        aoT = [actpool.tile([P, S], BF16, tag=f"aoT{t}", name=f"aoT{t}") for t in range(PAIRS)]

        # =================== filler steps ===================
        # Each step: (est_cycles, fn). Correctness never depends on pacing —
        # Tile syncs all data deps; pacing only shapes the PE timeline.

        def transpose_steps(xf, use_act):
            steps = []
            for sg in range(2):
                for ct in range(CT):
                    def fn(sg=sg, ct=ct, xf=xf, use_act=use_act):
                        tps = ps_fill.tile([P, 512], FP32, tag="qkps", name=f"tps{sg}_{ct}")
                        for j in range(4):
                            st = sg * 4 + j
                            nc.tensor.transpose(
                                out=tps[:, j * P:(j + 1) * P],
                                in_=xf[:, st, ct * P:(ct + 1) * P],
                                identity=ident,
                            )
                        if use_act:
                            nc.scalar.copy(
                                out=xT[ct][:, sg * 512:(sg + 1) * 512], in_=tps)
                        else:
                            nc.vector.tensor_copy(
                                out=xT[ct][:, sg * 512:(sg + 1) * 512], in_=tps)
                    steps.append((10 ** 6, 1100, fn))
            return steps

        def qk_half_steps(mt, nh, deadline=10 ** 6):
            """6 accumulation matmuls producing qk[mt] columns nh*512:+512.

            Tile links reads only to PRIOR writes, so every filler carries a
            hard deadline: the stage index before whose emission it MUST have
            been emitted (consumers emitted later then get real RAW edges).
            """
            steps = []
            box = {}
            for ct in range(CT):
                def fn(mt=mt, nh=nh, ct=ct, box=box):
                    if ct == 0:
                        box["ps"] = ps_fill.tile([P, 512], FP32, tag="qkps",
                                                 name=f"qkps{mt}_{nh}")
                    ps = box["ps"]
                    nc.tensor.matmul(
                        ps,
                        lhsT=wqkv[ct][:, mt * P:(mt + 1) * P],
                        rhs=xT[ct][:, nh * 512:(nh + 1) * 512],
                        start=(ct == 0), stop=(ct == CT - 1),
                    )
                    if ct == CT - 1:
                        nc.vector.tensor_scalar_add(
                            out=qk[mt][:, nh * 512:(nh + 1) * 512],
                            in0=ps, scalar1=qkb[:, mt:mt + 1],
                        )
                steps.append((deadline, 512, fn))
            return steps

        def v_steps(t, base=0):
            """v for head pair t (dims t*128..+128), all 8 seq tiles.
            4 seq tiles share one [P,512] PSUM tile (one quarter each).
            Deadline: PV(t, 0, kk) consumes vsb8[kk//2] head slices at stage
            base + kk."""
            steps = []
            box = {}
            for st in range(ST):
                def fn(t=t, st=st, box=box):
                    q = st % 4
                    if q == 0:
                        box["ps"] = ps_fill.tile([P, 512], FP32, tag="qkps",
                                                 name=f"vps{t}_{st}")
                    ps = box["ps"]
                    sl = slice(q * P, (q + 1) * P)
                    for ct in range(CT):
                        nc.tensor.matmul(
                            ps[:, sl],
                            lhsT=xT[ct][:, st * P:(st + 1) * P],
                            rhs=wqkv[ct][:, 2 * C + t * P: 2 * C + (t + 1) * P],
                            start=(ct == 0), stop=(ct == CT - 1),
                        )
                    nc.vector.tensor_add(
                        out=vsb8[st // 2][:, st % 2, 2 * t:2 * t + 2, 0:HD],
                        in0=ps[:, sl].rearrange("p (h d) -> p h d", d=HD),
                        in1=vb[:, t * P:(t + 1) * P].rearrange("p (h d) -> p h d", d=HD),
                    )
                steps.append((base + ((st | 1) if PV_DR else st), 800, fn))
            return steps

        def v_wide_steps(u, base):
            """v for pairs 2u,2u+1 together: 256-col streams halve the
            per-128-col Ldweights overhead; only for pairs 2..5 whose
            deadlines are far enough to absorb the coarser steps."""
            steps = []
            box = {}
            for st in range(ST):
                def fn(u=u, st=st, box=box):
                    if st % 2 == 0:
                        box["ps"] = ps_fill.tile([P, 512], FP32, tag="qkps",
                                                 name=f"vwps{u}_{st}")
                    ps = box["ps"]
                    sl = slice((st % 2) * 256, (st % 2) * 256 + 256)
                    for ct in range(CT):
                        nc.tensor.matmul(
                            ps[:, sl],
                            lhsT=xT[ct][:, st * P:(st + 1) * P],
                            rhs=wqkv[ct][:, 2 * C + u * 256: 2 * C + (u + 1) * 256],
                            start=(ct == 0), stop=(ct == CT - 1),
                        )
                    nc.vector.tensor_add(
                        out=vsb8[st // 2][:, st % 2, 4 * u:4 * u + 4, 0:HD],
                        in0=ps[:, sl].rearrange("p (h d) -> p h d", d=HD),
                        in1=vb[:, u * 256:(u + 1) * 256].rearrange("p (h d) -> p h d", d=HD),
                    )
                steps.append((base + ((st | 1) if PV_DR else st), 1600, fn))
            return steps

        def pcast_steps(deadline):
            steps = []
            for ct in range(CT):
                def fn(ct=ct):
                    nc.vector.tensor_copy(out=wproj[ct], in_=pstg[ct // 2][:, ct % 2, :])
                steps.append((deadline, 50, fn))
            return steps

        def proj_st_steps(st):
            """proj for seq tile st in column halves of 512 and 256."""
            steps = []
            box = {}
            for half, width in ((0, 512), (1, 256)):
                for ct in range(CT):
                    def fn(st=st, half=half, width=width, ct=ct, box=box):
                        off = half * 512
                        if ct == 0:
                            box[half] = ps_fill.tile([P, 512], FP32, tag="qkps",
                                                     name=f"prps{st}_{half}")
                        ps = box[half]
                        nc.tensor.matmul(
                            ps[:, 0:width],
                            lhsT=aoT[ct][:, st * P:(st + 1) * P],
                            rhs=wproj[ct][:, off:off + width],
                            start=(ct == 0), stop=(ct == CT - 1),
                        )
                        if ct == CT - 1:
                            ost = opool.tile([P, 512], FP32, tag="ostg", bufs=3,
                                             name=f"ostg{st}_{half}")
                            nc.vector.tensor_add(
                                out=ost[:, 0:width], in0=ps[:, 0:width],
                                in1=pb[:, off:off + width])
                            eng = (nc.scalar, nc.sync, nc.gpsimd)[(2 * st + half) % 3]
                            eng.dma_start(
                                out=bass.AP(tensor=out_ext,
                                            offset=st * P * C + off,
                                            ap=[[C, P], [1, width]]),
                                in_=ost[:, 0:width])
                    steps.append((10 ** 6, width, fn))
            return steps

        # =================== per-rep body ===================
        prev_tail = None
        xf_next = None
        for _rep in range(reps):
            first = _rep == 0
            last = _rep == reps - 1
            xf = xf0 if first else xf_next

            if first:
                for _, _, fn in transpose_steps(xf, use_act=True):
                    fn()
            if not last:
                xf_next = load_x(f"{_rep + 1}")

            # prefix: ONLY what gates scores(0)/exp(0) — pair-0's nh0 qk
            # fills. For rep>0 these were pre-emitted in the previous rep's
            # pair-5 feed (right after its transposes), so the new exp
            # stream starts almost immediately after the old window.
            vs0 = v_steps(0, base=0)
            if first:
                for _, _, fn in qk_half_steps(0, 0) + qk_half_steps(PAIRS, 0):
                    fn()

            # filler deque: (deadline_stage, cost, fn), consumed via pop()
            fillers = []
            fillers += vs0
            if prev_tail is not None:
                # previous rep's proj st4..7: early filler so the new exp
                # stream overlaps the old tail. Deadline 20 keeps it from
                # stalling this rep's deferred aoT writes (WAR edges).
                fillers += [(20, c, f) for _, c, f in prev_tail]
                prev_tail = None
            fillers += qk_half_steps(PAIRS, 1, deadline=3)
            fillers += qk_half_steps(0, 1, deadline=7)
            if first:
                fillers += pcast_steps(deadline=87)
            for t2 in range(1, PAIRS):
                base = 16 * t2
                a = (qk_half_steps(t2, 0, deadline=base - 1)
                     + qk_half_steps(t2, 1, deadline=base + 7))
                b = (qk_half_steps(PAIRS + t2, 0, deadline=base - 1)
                     + qk_half_steps(PAIRS + t2, 1, deadline=base + 3))
                if t2 == 1:
                    v = v_steps(t2, base=base)
                elif t2 in (2, 4):
                    v = v_wide_steps(t2 // 2, base=base)
                else:
                    v = []
                merged = []
                for i in range(12):
                    if i < len(v):
                        merged.append(v[i])
                    merged.append(a[i])
                    merged.append(b[i])
                fillers += merged
            fillers.sort(key=lambda s: s[0])
            fillers.reverse()

            # pair-5 qh=0 PE feed: the next rep's transposes, then its
            # prefix qk fills (safe: each step's PSUM reader is emitted
            # within the step, so the qkps ring never dangles).
            if not last:
                tp_next = transpose_steps(xf_next, use_act=False)
                tp_next += qk_half_steps(0, 0) + qk_half_steps(PAIRS, 0)
            else:
                tp_next = []
            tp_next.reverse()

            # proj fillers: only during pair-5 qh=1 stages
            proj_window = []
            for st in range(4):
                proj_window += proj_st_steps(st)
            proj_window.reverse()

            pending_muls = []
            if True:
                stages = [(t, qh, kk)
                          for t in range(PAIRS) for qh in range(2) for kk in range(ST)]
                sab_tiles = {}
                pt8_tiles = {}

                def emit_scores(s):
                    t, qh, kk = s
                    qsl = slice(qh * 512, (qh + 1) * 512)
                    ksl = slice(kk * P, (kk + 1) * P)
                    sAB = ps_attn.tile([P, S], FP32, tag="sAB", bufs=2,
                                       name=f"sAB{t}_{qh}_{kk}")
                    nc.tensor.matmul(
                        sAB[:, 0:512],
                        lhsT=qk[PAIRS + t][0:HD, ksl], rhs=qk[t][0:HD, qsl],
                        start=True, stop=True, tile_position=(0, 0),
                    )
                    nc.tensor.matmul(
                        sAB[:, 512:1024],
                        lhsT=qk[PAIRS + t][HD:P, ksl], rhs=qk[t][HD:P, qsl],
                        start=True, stop=True, tile_position=(HD, 0),
                    )
                    sab_tiles[s] = sAB

                emit_scores(stages[0])
                oA = oB = None
                for i, s in enumerate(stages):
                    t, qh, kk = s
                    last_pair = t == PAIRS - 1
                    # hard deadlines: anything this stage's emissions consume
                    # must already be emitted (Tile links reads only to prior
                    # writes — a late filler is a CORRECTNESS bug).
                    while fillers and fillers[-1][0] <= i:
                        fillers.pop()[2]()
                    sAB = sab_tiles.pop(s)
                    if kk % 2 == 0:
                        pt8 = ptpool.tile([P, 2, S], F8 if PV_DR else BF16,
                                          tag="pt8", bufs=3 if PV_DR else 2,
                                          name=f"pt8_{t}_{qh}_{kk}")
                        pt8_tiles[(t, qh, kk // 2)] = pt8
                    else:
                        pt8 = pt8_tiles.pop((t, qh, kk // 2))
                    nc.scalar.activation(
                        out=pt8[:, kk % 2, :], in_=sAB, func=Exp, scale=SCALE)
                    if i + 1 < len(stages):
                        emit_scores(stages[i + 1])
                    if PV_DR and kk % 2 == 1:
                        if kk == 1:
                            oA = ps_attn.tile([VM, 512], FP32, tag="oA", name=f"oA{t}_{qh}")
                            oB = ps_attn.tile([VM, 512], FP32, tag="oB", name=f"oB{t}_{qh}")
                        sp = kk // 2
                        nc.tensor.matmul(
                            oA, lhsT=vsb8[sp][:, :, 2 * t, :], rhs=pt8[:, :, 0:512],
                            start=(kk == 1), stop=(kk == ST - 1), perf_mode=DR,
                        )
                        nc.tensor.matmul(
                            oB, lhsT=vsb8[sp][:, :, 2 * t + 1, :], rhs=pt8[:, :, 512:1024],
                            start=(kk == 1), stop=(kk == ST - 1), perf_mode=DR,
                        )
                    elif not PV_DR:
                        if kk == 0:
                            oA = ps_attn.tile([VM, 512], FP32, tag="oA", name=f"oA{t}_{qh}")
                            oB = ps_attn.tile([VM, 512], FP32, tag="oB", name=f"oB{t}_{qh}")
                        sp = kk // 2
                        nc.tensor.matmul(
                            oA, lhsT=vsb8[sp][:, kk % 2, 2 * t, :], rhs=pt8[:, kk % 2, 0:512],
                            start=(kk == 0), stop=(kk == ST - 1),
                        )
                        nc.tensor.matmul(
                            oB, lhsT=vsb8[sp][:, kk % 2, 2 * t + 1, :], rhs=pt8[:, kk % 2, 512:1024],
                            start=(kk == 0), stop=(kk == ST - 1),
                        )
                    # drain fillers up to the stage quota; pair-5 has no
                    # later deadlines — drain its feeds (transposes/proj)
                    # aggressively so nothing spills past the window
                    if last_pair:
                        quota = 2900 if qh == 0 else 2600
                    else:
                        quota = QUOTA_ODD if kk % 2 else QUOTA_EVEN
                    acc = 0
                    while acc < quota:
                        if fillers:
                            src = fillers
                        elif last_pair and qh == 0 and tp_next:
                            src = tp_next
                        elif last_pair and qh == 1 and proj_window:
                            src = proj_window
                        else:
                            break
                        _, cost, fn = src.pop()
                        fn()
                        acc += cost

                    if kk != ST - 1:
                        continue
                    # (t, qh) slice complete: fast PSUM release + deferred norm
                    qsl = slice(qh * 512, (qh + 1) * 512)
                    uA = rpool.tile([P, 512], FP32, tag="uA", bufs=3)
                    uB = rpool.tile([P, 512], FP32, tag="uB", bufs=3)
                    nc.vector.tensor_copy(out=uA[0:HD + 1, :], in_=oA[0:HD + 1, :])
                    nc.vector.tensor_copy(out=uB[0:HD + 1, :], in_=oB[0:HD + 1, :])
                    uBs = rpool.tile([P, 512], FP32, tag="uBs", bufs=3)
                    if last_pair:
                        # fast-path: broadcast via K=1 PE matmul, not DMA
                        nc.vector.reciprocal(out=uA[HD:HD + 1, :], in_=uA[HD:HD + 1, :])
                        nc.vector.reciprocal(out=uB[HD:HD + 1, :], in_=uB[HD:HD + 1, :])
                        nc.sync.dma_start(out=uBs[HD:P, :], in_=uB[0:HD, :])
                        rbf = rpool.tile([P, 512], BF16, tag="rbf", bufs=1)
                        nc.vector.tensor_copy(out=rbf[HD:HD + 1, :], in_=uA[HD:HD + 1, :])
                        rbfB = rpool.tile([P, 512], BF16, tag="rbfB", bufs=1)
                        nc.vector.tensor_copy(out=rbfB[HD:HD + 1, :], in_=uB[HD:HD + 1, :])
                        rAp = ps_fill.tile([HD, 512], FP32, tag="qkps", name=f"rAp{qh}")
                        nc.tensor.matmul(
                            rAp, lhsT=ones_r[HD:HD + 1, 0:HD], rhs=rbf[HD:HD + 1, :],
                            start=True, stop=True, tile_position=(HD, 0),
                        )
                        rBp = ps_fill.tile([P, 512], FP32, tag="qkps", name=f"rBp{qh}")
                        nc.tensor.matmul(
                            rBp[HD:P, :], lhsT=ones_r[HD:HD + 1, 0:HD], rhs=rbfB[HD:HD + 1, :],
                            start=True, stop=True, tile_position=(HD, HD),
                        )
                        nc.vector.tensor_mul(
                            out=aoT[t][0:HD, qsl], in0=uA[0:HD, :], in1=rAp[0:HD, :]
                        )
                        nc.vector.tensor_mul(
                            out=aoT[t][HD:P, qsl], in0=uBs[HD:P, :], in1=rBp[HD:P, :]
                        )
                        if qh == 0:
                            # proj fillers read aoT[0..4] next: flush deferred muls
                            for m in pending_muls:
                                m()
                            pending_muls.clear()
                    else:
                        nc.vector.reciprocal(out=uA[HD:HD + 1, :], in_=uA[HD:HD + 1, :])
                        nc.vector.reciprocal(out=uB[HD:HD + 1, :], in_=uB[HD:HD + 1, :])
                        nc.gpsimd.dma_start(out=uBs[HD:P, :], in_=uB[0:HD, :])
                        rA = rpool.tile([P, 512], FP32, tag="rA", bufs=3)
                        rB = rpool.tile([P, 512], FP32, tag="rB", bufs=3)
                        dA = dscr.tile([512], FP32, tag="dA")
                        dB = dscr.tile([512], FP32, tag="dB")
                        nc.sync.dma_start(out=dA, in_=uA[HD:HD + 1, :])
                        nc.gpsimd.dma_start(out=dB, in_=uB[HD:HD + 1, :])
                        nc.sync.dma_start(
                            out=rA[0:HD, :],
                            in_=bass.AP(tensor=dA.tensor, offset=dA.offset, ap=[[0, HD], [1, 512]]),
                        )
                        nc.gpsimd.dma_start(
                            out=rB[HD:P, :],
                            in_=bass.AP(tensor=dB.tensor, offset=dB.offset, ap=[[0, HD], [1, 512]]),
                        )
                        def emit_muls(t=t, qsl=qsl, uA=uA, rA=rA, uBs=uBs, rB=rB):
                            nc.vector.tensor_mul(
                                out=aoT[t][0:HD, qsl], in0=uA[0:HD, :], in1=rA[0:HD, :]
                            )
                            nc.vector.tensor_mul(
                                out=aoT[t][HD:P, qsl], in0=uBs[HD:P, :], in1=rB[HD:P, :]
                            )
                        pending_muls.append(emit_muls)
                        if len(pending_muls) > 2:
                            pending_muls.pop(0)()

            for m in pending_muls:
                m()
            pending_muls.clear()
            # leftovers (if quotas underdrained — correctness, not perf)
            for lst in (fillers, tp_next, proj_window):
                while lst:
                    lst.pop()[2]()

            prev_tail = []
            for st in range(4, ST):
                prev_tail += proj_st_steps(st)

        for _, _, fn in prev_tail:
            fn()

    split_multiwait(nc)
    return nc


_NC_CACHE = None


def get_nc():
    global _NC_CACHE
    if _NC_CACHE is None:
        _NC_CACHE = build_nc()
    return _NC_CACHE


def kernel(x, qkv_w, qkv_b, proj_w, proj_b):
    x = np.ascontiguousarray(np.asarray(x, dtype=np.float32))
    in_common = {
        "qkv_w": np.ascontiguousarray(np.asarray(qkv_w, dtype=np.float32)),
        "qkv_b": np.ascontiguousarray(np.asarray(qkv_b, dtype=np.float32)),
        "proj_w": np.ascontiguousarray(np.asarray(proj_w, dtype=np.float32)),
        "proj_b": np.ascontiguousarray(np.asarray(proj_b, dtype=np.float32)),
    }
    in_maps = [{"x": x[b], **in_common} for b in range(N_CORES)]
    nc = get_nc()
    res = run_bass_kernel_spmd(nc, in_maps, core_ids=list(range(N_CORES)))
    return np.stack([res.results[b]["out"] for b in range(N_CORES)], axis=0)
